# revision 46
# baseline (speedup 1.0000x reference)
# Multi-head causal attention (B=4, T=2048, D=1024, H=16) on 8 TRN2 NeuronCores.
#
# Sharding: data-parallel over the 4 batches x tensor-parallel over heads
# (each core of a batch pair owns 8 of the 16 heads: the matching 512-column
# slices of Wq/Wk/Wv and 512-row slice of Wo). Each core computes its heads'
# K/V/Q for the full sequence and a PARTIAL output projection; the host sums
# the two partials per batch (the "all-reduce after Wo" runs on the host
# during unsharding), so there are NO on-device collectives and no
# duplicated projection work.
#
# Per-core kernel (all matmul operands bf16, fp32 PSUM accumulation):
#   kT/qT = W @ x^T per head-pair group, V kept (t, d)-major with an extra
#   ones column so the attention-value matmul also produces the softmax
#   denominator. Scores are computed transposed (tk partition, tq free),
#   exp on the Scalar engine (no max subtraction: |scores| <= ~3), causal
#   masking via one multiplicative {0,1} mask tile (identical on all cores:
#   query blocks are contiguous, so only the diagonal kb pair is masked),
#   denominator broadcast across partitions on GpSimd.
import os
import numpy as np
import ml_dtypes

# a crashed prior run can leave the cores in a latched slow state
# (~20% degraded); a core reset at runtime init clears it
os.environ.setdefault("NEURON_RT_RESET_CORES", "1")

B, T, D, H, DH, P = 4, 2048, 1024, 16, 64, 128
HC = 512           # head-slice width per core (8 heads x 64)
G = 4              # head-pair groups per core
NCORES = 8
BF16 = ml_dtypes.bfloat16

_COMPILED = {}


_EDGE = False


def _build_nc():
    from contextlib import ExitStack
    import concourse.mybir as mybir
    import concourse.tile as tile
    from concourse import bacc

    bf = mybir.dt.bfloat16
    f32 = mybir.dt.float32
    EXP = mybir.ActivationFunctionType.Exp

    nc = bacc.Bacc("TRN2", target_bir_lowering=False, debug=False,
                   num_devices=NCORES)

    # ---- DRAM I/O ----
    xk_d = nc.dram_tensor("xk", [D, T], bf, kind="ExternalInput").ap()
    wq_d = nc.dram_tensor("wqT", [D, HC], bf, kind="ExternalInput").ap()
    wk_d = nc.dram_tensor("wkT", [D, HC], bf, kind="ExternalInput").ap()
    wv_d = nc.dram_tensor("wvT", [D, HC], bf, kind="ExternalInput").ap()
    wo_d = nc.dram_tensor("woT", [HC, D], bf, kind="ExternalInput").ap()
    bq_d = nc.dram_tensor("bq_r", [P, G], f32, kind="ExternalInput").ap()
    bk_d = nc.dram_tensor("bk_r", [P, G], f32, kind="ExternalInput").ap()
    bo_d = nc.dram_tensor("bo_r", [P, 8], f32, kind="ExternalInput").ap()
    bv_d = nc.dram_tensor("bv_bc", [P, HC], bf, kind="ExternalInput").ap()
    cm_d = nc.dram_tensor("cmask", [P, 384 if _EDGE else 1024], bf, kind="ExternalInput").ap()
    y_d = nc.dram_tensor("yT", [D, T], f32, kind="ExternalOutput").ap()
    import os
    _dbg = os.environ.get("KDBG") == "1"
    if _dbg:
        kdbg_d = nc.dram_tensor("kdbg", [P, G, T], bf, kind="ExternalOutput").ap()
        qdbg_d = nc.dram_tensor("qdbg", [P, G, T], bf, kind="ExternalOutput").ap()
        vdbg_d = nc.dram_tensor("vdbg", [P, 16, 8, 65], bf, kind="ExternalOutput").ap()
        adbg_d = nc.dram_tensor("adbg", [P, G, T], bf, kind="ExternalOutput").ap()

    xk_r = xk_d.rearrange("(g p) t -> p g t", p=P)

    with tile.TileContext(nc) as tc, ExitStack() as ctx:
        const = ctx.enter_context(tc.tile_pool(name="const", bufs=1))
        xchunk = ctx.enter_context(tc.tile_pool(name="xchunk", bufs=3))
        expps = ctx.enter_context(tc.tile_pool(name="expps", bufs=6))
        small = ctx.enter_context(tc.tile_pool(name="small", bufs=2))
        ps_s = ctx.enter_context(tc.tile_pool(name="ps_s", bufs=2, space="PSUM"))
        ps_av = ctx.enter_context(tc.tile_pool(name="ps_av", bufs=1, space="PSUM"))
        ps_m = ctx.enter_context(tc.tile_pool(name="ps_m", bufs=2, space="PSUM"))

        # ---- resident SBUF tensors ----
        wq_sb = const.tile([P, 8, HC], bf)       # [k, kg, (g dout)]
        wk_sb = const.tile([P, 8, HC], bf)
        wv_sb = const.tile([P, 8, HC], bf)       # [k, kg, d] (moving operand)
        wo_sb = const.tile([P, G, D], bf)        # [d, g, (o dout)]
        k_sb = const.tile([P, G, T], bf)         # k^T per head-pair group
        q_sb = const.tile([P, G, T], bf)
        v1_sb = const.tile([P, 16, 8, 65], bf)   # [tk, tkblk, head, V|1]
        a_sb = const.tile([P, G, T], bf)         # attention out (d, tq)
        mk_sb = const.tile([P, 384 if _EDGE else 1024], bf)
        bq_sb = const.tile([P, G], f32)
        bk_sb = const.tile([P, G], f32)
        bo_sb = const.tile([P, 8], f32)
        bv_sb = const.tile([P, HC], bf)

        dma = nc.sync.dma_start
        nc.vector.memset(v1_sb[:, :, :, 64:65], 1.0)

        mm = nc.tensor.matmul

        def _v_group(xc, ts, ti):
            t = 4 * ts + ti
            ps = ps_m.tile([P, HC], f32, name="ps", tag="ps")
            for kg in range(8):
                mm(ps, lhsT=xc[:, kg, ti * P:(ti + 1) * P],
                   rhs=wv_sb[:, kg, :],
                   start=(kg == 0), stop=(kg == 7))
            nc.vector.tensor_add(
                out=v1_sb[:, t, :, 0:64],
                in0=ps.rearrange("p (h c) -> p h c", c=64),
                in1=bv_sb.rearrange("p (h c) -> p h c", c=64))

        def _k_group(xc, ts, g):
            ps = ps_m.tile([P, 512], f32, name="ps", tag="ps")
            for kg in range(8):
                mm(ps, lhsT=wk_sb[:, kg, g * P:(g + 1) * P], rhs=xc[:, kg, :],
                   start=(kg == 0), stop=(kg == 7))
            nc.vector.tensor_scalar_add(
                out=k_sb[:, g, ts * 512:(ts + 1) * 512], in0=ps,
                scalar1=bk_sb[:, g:g + 1])

        def _q_group(xc, ts, g):
            ps = ps_m.tile([P, 512], f32, name="ps", tag="ps")
            for kg in range(8):
                mm(ps, lhsT=wq_sb[:, kg, g * P:(g + 1) * P], rhs=xc[:, kg, :],
                   start=(kg == 0), stop=(kg == 7))
            nc.vector.tensor_scalar_add(
                out=q_sb[:, g, ts * 512:(ts + 1) * 512], in0=ps,
                scalar1=bq_sb[:, g:g + 1])

        def proj_chunk_thunks(ts, split=False):
            # stream 512 tokens of x^T; V/K/Q projection groups returned as
            # thunks so they interleave with attention emission. split=True
            # issues the load as two half-chunks so the first V matmuls can
            # start as soon as the first 0.5 MB lands.
            from functools import partial
            xc = xchunk.tile([P, 8, 512], bf, name="xc", tag="xc")
            if split:
                dma(out=xc[:, 0:4, :],
                    in_=xk_r[:, 0:4, ts * 512:(ts + 1) * 512])
                split()
                dma(out=xc[:, 4:8, :],
                    in_=xk_r[:, 4:8, ts * 512:(ts + 1) * 512])
            else:
                dma(out=xc, in_=xk_r[:, :, ts * 512:(ts + 1) * 512])
            th = [partial(_v_group, xc, ts, ti) for ti in range(4)]
            th += [partial(_k_group, xc, ts, g) for g in range(G)]
            th += [partial(_q_group, xc, ts, g) for g in range(G)]
            return th

        def attn_slot(g, j, pend=None):
            # one accumulator bank PER HEAD: PSUM accumulation groups must be
            # bank-aligned on HW (a group at a 256-col offset inside a bank
            # silently corrupts). Row 64 collects the softmax denominator
            # via the ones column of v1_sb. Slot j covers contiguous query
            # blocks (2j, 2j+1); only the kk == j iteration needs masking,
            # and there the kb 2j+1 block is upper-triangular for q-block 2j
            # (all dead), so it is computed for q-block 2j+1 only: the edge
            # tile is [kb0(q0 q1) | kb1 q1] x 2 heads = 768 columns.
            # One pending projection/Wo thunk drains per kk so the PE stays
            # fed (and frequency-warm) while the ScalarE exp chain paces the
            # slot.
            pav = [ps_av.tile([65, 256], f32, tag=f"pav{c}",
                              name=f"pav{c}") for c in (0, 1)]
            last_k = 2 * j + 1
            for kk in range(j + 1):
                kb = (2 * kk, 2 * kk + 1)
                edge = (kk == j) and _EDGE
                iw = (256, 128) if edge else (256, 256)
                qo = (0, 128) if edge else (0, 0)
                ps = ps_s.tile([P, 1024], f32, name="scps", tag="scps")
                expp = expps.tile([P, 1024], bf, name="expp", tag="expp")
                for c in (0, 1):         # head within pair
                    for i in (0, 1):     # tk block within pair
                        mm(ps[:, c * 512 + i * 256: c * 512 + i * 256 + iw[i]],
                           lhsT=k_sb[64 * c:64 * c + 64, g,
                                     kb[i] * P:(kb[i] + 1) * P],
                           rhs=q_sb[64 * c:64 * c + 64, g,
                                    j * 256 + qo[i]:(j + 1) * 256],
                           start=True, stop=True,
                           tile_position=(64 * c, 0))
                if edge:
                    # dead zone at [384:512]/[896:1024] is neither exp'd nor
                    # read; per-head 384-wide exp + mask
                    for c in (0, 1):
                        nc.scalar.activation(
                            out=expp[:, c * 512:c * 512 + 384],
                            in_=ps[:, c * 512:c * 512 + 384],
                            func=EXP, scale=0.125)
                        nc.vector.tensor_mul(
                            expp[:, c * 512:c * 512 + 384],
                            expp[:, c * 512:c * 512 + 384], mk_sb)
                else:
                    nc.scalar.activation(out=expp, in_=ps, func=EXP,
                                         scale=0.125)
                    if kk == j:
                        nc.vector.tensor_mul(expp, expp, mk_sb)
                for c in (0, 1):
                    for i in (0, 1):
                        mm(pav[c][:, qo[i]:256],
                           lhsT=v1_sb[:, kb[i], 2 * g + c, :],
                           rhs=expp[:, c * 512 + i * 256:
                                    c * 512 + i * 256 + iw[i]],
                           start=(kb[i] == 0), stop=(kb[i] == last_k),
                           skip_group_check=True)
                if pend:
                    drain(pend, 1)
            # copy accumulators to SBUF right away so the PSUM banks free up
            # for the next slot; normalize runs off the PE critical path
            av = [small.tile([65, 256], f32, tag=f"av{c}", bufs=2,
                             name=f"av{c}") for c in (0, 1)]
            for c in (0, 1):
                nc.vector.tensor_copy(out=av[c], in_=pav[c])
            # both heads' denominators into one partition-base-0 tile
            # (reciprocal_approx_fast corrupts base!=0 inputs on HW)
            den2 = small.tile([1, 512], f32, tag="den2", bufs=1, name="den2")
            for c in (0, 1):
                nc.vector.tensor_copy(out=den2[:, c * 256:(c + 1) * 256],
                                      in_=av[c][64:65, :])
            rec = small.tile([1, 512], f32, tag="rec", bufs=1, name="rec")
            nc.vector.reciprocal_approx_fast(out=rec, in_=den2)
            sbb = small.tile([64, 512], f32, tag="sbb", name="sbb")
            nc.gpsimd.partition_broadcast(sbb, rec)
            for c in (0, 1):
                nc.vector.tensor_mul(
                    out=a_sb[64 * c:64 * c + 64, g, j * 256:(j + 1) * 256],
                    in0=av[c][0:64, :], in1=sbb[:, c * 256:(c + 1) * 256])

        y_r = y_d.rearrange("(o p) t -> p o t", p=P)

        def wo_group(q4, o, w=1, ysb=None):
            # partial output-projection chunk (w*256 wide): contraction over
            # this core's 512 dims (4 groups); needs attn slots q4..q4+w-1.
            # Results collect in a shared per-chunk tile (one y DMA per chunk
            # instead of one per o-group: descriptor generation on the sync
            # engine is ~0.5 us per DMA).
            ps = ps_m.tile([P, 512], f32, name="ps", tag="ps")
            for kg in range(G):
                mm(ps[:, 0:w * 256], lhsT=wo_sb[:, kg, o * P:(o + 1) * P],
                   rhs=a_sb[:, kg, q4 * 256:(q4 + w) * 256],
                   start=(kg == 0), stop=(kg == 3))
            if ysb is None:
                ysb = small.tile([P, 512], f32, tag="ysb", name="ysb")
                nc.vector.tensor_scalar_add(out=ysb[:, 0:w * 256],
                                            in0=ps[:, 0:w * 256],
                                            scalar1=bo_sb[:, o:o + 1])
                dma(out=y_d[o * P:(o + 1) * P, q4 * 256:(q4 + w) * 256],
                    in_=ysb[:, 0:w * 256])
            else:
                nc.vector.tensor_scalar_add(out=ysb[:, o, :],
                                            in0=ps[:, 0:w * 256],
                                            scalar1=bo_sb[:, o:o + 1])

        def wo_chunk_thunks(q4, w=1):
            # all 8 o-groups of one (w*256)-token chunk + a single fused DMA
            ysb = small.tile([P, 8, w * 256], f32, tag=f"ych{w}",
                             name="ych", bufs=2)
            th = [(lambda o=o: wo_group(q4, o, w=w, ysb=ysb))
                  for o in range(8)]
            th.append(lambda: dma(out=y_r[:, :, q4 * 256:(q4 + w) * 256],
                                  in_=ysb))
            return th

        def drain(pend, n):
            for _ in range(min(n, len(pend))):
                pend.pop(0)()

        # Emission order sets PE priority: the attention inner loop is paced
        # by the ScalarE exp chain, so feed the PE projection/output-proj
        # groups BETWEEN attention slots to keep it busy (and HAM-warm).
        # DMA issue order = queue priority: x chunk 0 and wv go first so the
        # first V matmul isn't parked behind the other weight loads.
        wv_r = wv_d.rearrange("(kg p) d -> p kg d", p=P)

        def _wv_first_half():
            dma(out=wv_sb[:, 0:4, :], in_=wv_r[:, 0:4, :])
            dma(out=bv_sb, in_=bv_d)

        pend0 = proj_chunk_thunks(0, split=_wv_first_half)
        dma(out=wv_sb[:, 4:8, :], in_=wv_r[:, 4:8, :])
        drain(pend0, 4)                        # V-proj of chunk 0
        dma(out=wk_sb, in_=wk_d.rearrange("(kg p) d -> p kg d", p=P))
        dma(out=bk_sb, in_=bk_d)
        drain(pend0, 4)                        # K-proj of chunk 0
        dma(out=wq_sb, in_=wq_d.rearrange("(kg p) d -> p kg d", p=P))
        dma(out=bq_sb, in_=bq_d)
        drain(pend0, 4)                        # Q-proj of chunk 0
        dma(out=mk_sb, in_=cm_d)
        dma(out=bo_sb, in_=bo_d)

        # attn slot (g, j) needs K/V token blocks 0..2j+1 and Q block pair j,
        # i.e. chunks 0..ceil((2j+2)/4)-1: j=0,1 -> chunk 0+, j=2,3 -> 0..1,
        # j=4,5 -> 0..2, j=6,7 -> 0..3. Wo chunk q4 needs attention slot q4
        # of all groups. Chunk c's projections drain into phase j=c-1.
        pend = proj_chunk_thunks(1)
        for g in range(G):
            attn_slot(g, 0)
            drain(pend, 3)
        dma(out=wo_sb, in_=wo_d.rearrange("(g p) d -> p g d", p=P))
        pend = proj_chunk_thunks(2)
        for g in range(G):
            attn_slot(g, 1)
            drain(pend, 3)
        # late phases are paced by the ScalarE exp chain (the per-phase PE
        # deficit grows with j), so wo chunks drain as LATE as their slot
        # dependencies allow and the chunk-3 projections spread over
        # phases 3-4
        c3 = proj_chunk_thunks(3)
        p01 = wo_chunk_thunks(0, w=2)
        s4 = wo_chunk_thunks(4)
        wo_phase = {
            2: p01[:4],
            3: p01[4:] + c3[:2],
            4: c3[2:7],
            5: c3[7:] + s4[:1],
            6: s4[1:] + wo_chunk_thunks(2, w=2),
            7: wo_chunk_thunks(5) + wo_chunk_thunks(6),
        }
        for j in range(2, 8):
            pend = wo_phase[j]
            n = min(3, (len(pend) + 3) // 4)
            for g in range(G):
                attn_slot(g, j)
                drain(pend, n)
            drain(pend, 99)
        # final chunk: only the kg=3 matmul depends on the last attention
        # slot's a_sb, so the kg 0-2 partial sums for six o-groups pre-run
        # (in the by-then-free score/proj PSUM banks) while slot (3,7)
        # normalizes; the kg=3 taps + the last two o-groups follow.
        w7ps = []
        for _ in (0, 1):
            t = ps_s.tile([P, 1024], f32, name="scps", tag="scps")
            w7ps += [t[:, 0:256], t[:, 512:768]]
        for _ in (0, 1):
            t = ps_m.tile([P, 512], f32, name="ps", tag="ps")
            w7ps.append(t[:, 0:256])
        for o in range(6):
            for kg in range(3):
                mm(w7ps[o], lhsT=wo_sb[:, kg, o * P:(o + 1) * P],
                   rhs=a_sb[:, kg, 1792:2048],
                   start=(kg == 0), stop=False, skip_group_check=True)
        for o in range(6):
            mm(w7ps[o], lhsT=wo_sb[:, 3, o * P:(o + 1) * P],
               rhs=a_sb[:, 3, 1792:2048],
               start=False, stop=True, skip_group_check=True)
            ysb = small.tile([P, 512], f32, tag="ysb", name="ysb")
            nc.vector.tensor_scalar_add(out=ysb[:, 0:256], in0=w7ps[o],
                                        scalar1=bo_sb[:, o:o + 1])
            dma(out=y_d[o * P:(o + 1) * P, 1792:2048], in_=ysb[:, 0:256])
        for o in (6, 7):
            wo_group(7, o)
    nc.compile()
    return nc


def _get_nc():
    if "nc" not in _COMPILED:
        _COMPILED["nc"] = _build_nc()
    return _COMPILED["nc"]


def _mask():
    # diagonal kb pair for slot j: kb 2j (tri for q-block 2j, full for
    # q-block 2j+1), kb 2j+1 (zero for q-block 2j, tri for q-block 2j+1).
    # expp layout: [kb0 h0 | kb1 h0 | kb0 h1 | kb1 h1], each 256 = [q0|q1].
    tri = np.triu(np.ones((P, P), np.float32))   # keep tk <= tq
    on = np.ones((P, P), np.float32)
    if _EDGE:
        # kb0 vs (q0, q1), kb1 vs q1 — per-head [P, 384] pattern
        return np.concatenate([tri, on, tri], axis=1).astype(BF16)
    z = np.zeros((P, P), np.float32)
    m0 = np.concatenate([tri, on], axis=1)
    m1 = np.concatenate([z, tri], axis=1)
    return np.concatenate([m0, m1, m0, m1], axis=1).astype(BF16)


def _make_in_maps(x, wq, bq, wk, bk, wv, bv, wo, bo):
    bfc = lambda a: np.ascontiguousarray(np.asarray(a, np.float32).T).astype(BF16)
    wqT, wkT, wvT = bfc(wq), bfc(wk), bfc(wv)    # [D_in, D_out]
    woT = bfc(wo)                                # [D_in(=heads), D_out]
    mask = _mask()
    bias_r = lambda b, h: np.ascontiguousarray(
        np.asarray(b, np.float32)[h * HC:(h + 1) * HC].reshape(G, P).T)
    bo_full = np.ascontiguousarray(np.asarray(bo, np.float32).reshape(8, P).T)
    bo_zero = np.zeros((P, 8), np.float32)
    in_maps = []
    for core in range(NCORES):
        b, h = core // 2, core % 2
        hs = slice(h * HC, (h + 1) * HC)
        xT = np.ascontiguousarray(np.asarray(x[b], np.float32).T).astype(BF16)
        m = {
            "xk": xT,
            "wqT": np.ascontiguousarray(wqT[:, hs]),
            "wkT": np.ascontiguousarray(wkT[:, hs]),
            "wvT": np.ascontiguousarray(wvT[:, hs]),
            "woT": np.ascontiguousarray(woT[hs, :]),
            "bq_r": bias_r(bq, h),
            "bk_r": bias_r(bk, h),
            "bv_bc": np.ascontiguousarray(np.broadcast_to(
                np.asarray(bv, np.float32)[hs].astype(BF16), (P, HC))),
            "bo_r": bo_full if h == 0 else bo_zero,
            "cmask": mask,
        }
        in_maps.append(m)
    return in_maps


def _run(inputs, trace=False):
    from concourse.bass_utils import run_bass_kernel_spmd
    nc = _get_nc()
    in_maps = _make_in_maps(**inputs)
    res = run_bass_kernel_spmd(nc, in_maps, list(range(NCORES)), trace=trace)
    y = np.empty((B, T, D), np.float32)
    for b in range(B):
        y[b] = (res.results[2 * b]["yT"] + res.results[2 * b + 1]["yT"]).T
    return y, res


def kernel(**inputs):
    y, _ = _run(inputs, trace=False)
    return y
